# revision 1
# baseline (speedup 1.0000x reference)
"""Trainium2 Bass kernel for a ViT-style transformer block.

Reference semantics (B=16, N=577, D=768, H=12, DH=64, HID=3072):
    h   = LN(x) ; qkv = h @ qkv_w + qkv_b
    dp  = q k^T / sqrt(DH)           (per head)
    attn= softmax over the HEAD axis (axis=1 of (B,H,N,N) -- unusual!)
    x   = x + (attn @ v) @ proj_w + proj_b
    h   = LN(x); x = x + gelu(h @ fc1_w + fc1_b) @ fc2_w + fc2_b

Distribution: pure data parallelism -- 16 batches over 8 NeuronCores,
2 batches per core, full weights replicated, no collectives.

Per-core strategy: activations flow feature-major ([feat, token]) through
the matmuls, tokens padded 577->640 (5 tiles of 128).  Matmuls run in
bf16 (fp32 is 1/4 rate on the PE); LN / softmax-sum / residuals in fp32.
The head-axis softmax is exp(s)/sum_h exp(s) elementwise (scores are
O(1), so no max-subtraction is needed).

HW quirks found empirically (crash / silent corruption) and avoided here:
  * matmuls whose operands carry nonzero free offsets must not mix
    tile_position row strips -> q/k live in a [64, 24, 640] base-0 tile
    and every scores matmul contracts K=64 at tile_position (0,0).
  * the moving operand (rhs) of a matmul must be contiguous per
    partition -> attention weights are stored per-head contiguous
    ([k, kt, h, q]) so the attn@v rhs is an unstrided [128, q] slice.
"""

import sys
import time

if "/opt/trn_rl_repo" not in sys.path:
    sys.path.insert(0, "/opt/trn_rl_repo")

import numpy as np

B, N, D = 16, 577, 768
H, DH, HID = 12, 64, 3072
EPS = 1e-6
NCORES = 8
BPC = B // NCORES          # batches per core
P = 128
NT = 5                     # token tiles (640 padded)
NP = NT * P                # 640
DT = D // P                # 6
HT = HID // P              # 24
LAST_ROWS = N - 4 * P      # 65

LAST_EXEC_NS = None

_BUILT = {}


def _build(flags):
    """Build the single-core Bass program.  `flags` marks which optional
    affine parameters are non-trivial (biases nonzero / gains != 1)."""
    from contextlib import ExitStack

    import concourse.bass as bass
    from concourse import bacc
    import concourse.mybir as mybir
    import concourse.tile as tile
    from concourse.bass import ts, ds

    f32 = mybir.dt.float32
    bf16 = mybir.dt.bfloat16
    AF = mybir.ActivationFunctionType
    OP = mybir.AluOpType

    nc = bacc.Bacc(trn_type="TRN2", target_bir_lowering=False, debug=False,
                   enable_asserts=False)

    x_d = nc.dram_tensor("x", [BPC, N, D], f32, kind="ExternalInput").ap()
    qkvw_d = nc.dram_tensor("qkv_w", [D, 3 * D], f32, kind="ExternalInput").ap()
    qkvb_d = nc.dram_tensor("qkv_b", [3 * D], f32, kind="ExternalInput").ap()
    projw_d = nc.dram_tensor("proj_w", [D, D], f32, kind="ExternalInput").ap()
    projb_d = nc.dram_tensor("proj_b", [D], f32, kind="ExternalInput").ap()
    ln1g_d = nc.dram_tensor("ln1_g", [D], f32, kind="ExternalInput").ap()
    ln1b_d = nc.dram_tensor("ln1_b", [D], f32, kind="ExternalInput").ap()
    ln2g_d = nc.dram_tensor("ln2_g", [D], f32, kind="ExternalInput").ap()
    ln2b_d = nc.dram_tensor("ln2_b", [D], f32, kind="ExternalInput").ap()
    fc1w_d = nc.dram_tensor("fc1_w", [D, HID], f32, kind="ExternalInput").ap()
    fc1b_d = nc.dram_tensor("fc1_b", [HID], f32, kind="ExternalInput").ap()
    fc2w_d = nc.dram_tensor("fc2_w", [HID, D], f32, kind="ExternalInput").ap()
    fc2b_d = nc.dram_tensor("fc2_b", [D], f32, kind="ExternalInput").ap()
    out_d = nc.dram_tensor("out", [BPC, N, D], f32, kind="ExternalOutput").ap()

    def bcast(src1d):
        return bass.AP(tensor=src1d.tensor, offset=src1d.offset,
                       ap=[[0, P], src1d.ap[0]])

    with tile.TileContext(nc) as tc:
        with ExitStack() as ctx:
            # ---------------- resident weights (bf16) ----------------
            wpool = ctx.enter_context(tc.tile_pool(name="weights", bufs=1))
            singles = ctx.enter_context(tc.tile_pool(name="singles", bufs=1))

            qkvw = wpool.tile([P, DT, 3 * D], bf16)
            projw = wpool.tile([P, DT, D], bf16)
            fc1w = wpool.tile([P, DT, HID], bf16)
            fc2w = wpool.tile([P, HT, D], bf16)

            cast_engines = [nc.vector, nc.scalar, nc.gpsimd]
            n_cast = 0

            with tc.tile_pool(name="wstage", bufs=3) as wstage:
                def load_w(dst, src, kt_count, ncols):
                    nonlocal n_cast
                    for k in range(kt_count):
                        st = wstage.tile([P, HID], f32, tag="wst")
                        nc.sync.dma_start(st[:, :ncols],
                                          src[k * P:(k + 1) * P, :])
                        eng = cast_engines[n_cast % 3]
                        n_cast += 1
                        if eng is nc.scalar:
                            eng.activation(dst[:, k, :], st[:, :ncols], AF.Copy)
                        else:
                            eng.tensor_copy(dst[:, k, :], st[:, :ncols])

                load_w(qkvw, qkvw_d, DT, 3 * D)
                load_w(projw, projw_d, DT, D)
                load_w(fc1w, fc1w_d, DT, HID)
                load_w(fc2w, fc2w_d, HT, D)

            eps_t = singles.tile([P, 1], f32)
            nc.vector.memset(eps_t, EPS)

            qkvb = fc1b = None
            ln1g_r = ln1b_r = ln2g_r = ln2b_r = None
            projb_r = fc2b_r = vb_r = None
            if flags["qkv_b"]:
                qkvb = singles.tile([P, 2 * DT], f32)
                nc.sync.dma_start(
                    qkvb, qkvb_d[:2 * DT * P].rearrange("(t p) -> p t", p=P))
                vb_r = singles.tile([P, D], f32)
                nc.gpsimd.dma_start(vb_r, bcast(qkvb_d[2 * D:]))
            if flags["fc1_b"]:
                fc1b = singles.tile([P, HT], f32)
                nc.sync.dma_start(fc1b, fc1b_d.rearrange("(t p) -> p t", p=P))
            for fl, nmd in (("ln1_g", ln1g_d), ("ln1_b", ln1b_d),
                            ("ln2_g", ln2g_d), ("ln2_b", ln2b_d),
                            ("proj_b", projb_d), ("fc2_b", fc2b_d)):
                if flags[fl]:
                    t_ = singles.tile([P, D], f32, name=f"r_{fl}")
                    nc.gpsimd.dma_start(t_, bcast(nmd))
                    if fl == "ln1_g":
                        ln1g_r = t_
                    elif fl == "ln1_b":
                        ln1b_r = t_
                    elif fl == "ln2_g":
                        ln2g_r = t_
                    elif fl == "ln2_b":
                        ln2b_r = t_
                    elif fl == "proj_b":
                        projb_r = t_
                    else:
                        fc2b_r = t_

            # ---------------- activation pools ----------------
            o1pool = ctx.enter_context(tc.tile_pool(name="o1p", bufs=1))
            hpool = ctx.enter_context(tc.tile_pool(name="hp", bufs=2))
            statpool = ctx.enter_context(tc.tile_pool(name="stat", bufs=4))
            hTpool = ctx.enter_context(tc.tile_pool(name="hTp", bufs=1))
            qkpool = ctx.enter_context(tc.tile_pool(name="qkp", bufs=1))
            vpool = ctx.enter_context(tc.tile_pool(name="vp", bufs=1))
            epool = ctx.enter_context(tc.tile_pool(name="ep", bufs=1))
            spool = ctx.enter_context(tc.tile_pool(name="sp", bufs=1))
            wapool = ctx.enter_context(tc.tile_pool(name="wap", bufs=1))
            ghpool = ctx.enter_context(tc.tile_pool(name="ghp", bufs=1))
            fopool = ctx.enter_context(tc.tile_pool(name="fop", bufs=1))

            psb = ctx.enter_context(tc.tile_pool(name="psb", bufs=2,
                                                 space="PSUM"))
            pssc = ctx.enter_context(tc.tile_pool(name="pssc", bufs=2,
                                                  space="PSUM"))
            psav = ctx.enter_context(tc.tile_pool(name="psav", bufs=1,
                                                  space="PSUM"))

            def layer_norm(src, dst, g_r, b_r):
                """src [P, D] f32 -> dst [P, D] bf16 (normalized * g + b)."""
                stats = statpool.tile([P, 3, 6], f32, tag="bn")
                for c in range(3):
                    nc.vector.bn_stats(stats[:, c, :],
                                       src[:, c * 256:(c + 1) * 256])
                mv = statpool.tile([P, 2], f32, tag="mv")
                nc.vector.bn_aggr(mv, stats)
                std = statpool.tile([P, 1], f32, tag="std")
                nc.scalar.activation(std, mv[:, 1:2], AF.Sqrt, bias=eps_t)
                nc.vector.reciprocal(std, std)
                nc.vector.tensor_scalar(dst, src, mv[:, 0:1], std,
                                        op0=OP.subtract, op1=OP.mult)
                if g_r is not None:
                    nc.vector.tensor_tensor(dst, dst, g_r, OP.mult)
                if b_r is not None:
                    nc.vector.tensor_tensor(dst, dst, b_r, OP.add)

            for b in range(BPC):
                # ---- LN1 (x streamed per token tile) + transpose ----
                hT = hTpool.tile([P, DT, NP], bf16, tag="hT")
                for t in range(NT):
                    rows = P if t < NT - 1 else LAST_ROWS
                    xs = hpool.tile([P, D], f32, tag="xs")
                    if rows < P:
                        nc.vector.memset(xs, 0.0)
                    nc.sync.dma_start(xs[:rows, :], x_d[b, ds(t * P, rows), :])
                    h_t = hpool.tile([P, D], bf16, tag="h")
                    layer_norm(xs, h_t, ln1g_r, ln1b_r)
                    for dt in range(DT):
                        nc.sync.dma_start_transpose(
                            hT[:, dt, ts(t, P)], h_t[:, ts(dt, P)])
                # zero padded token columns: q/k/v of pad tokens become 0
                nc.vector.memset(hT[:, :, N:], 0.0)

                # ---- QKV ----
                # q/k: feature-major, all 24 head-slots on partitions 0-63
                # (base-0 so scores matmuls never mix tile_position strips)
                qk = qkpool.tile([64, 2 * H, NP], bf16)
                for do in range(2 * DT):          # 12 tiles x 128 = q,k douts
                    is_k = do >= DT
                    for ncn in range(2):
                        ps = psb.tile([P, 512], f32, tag="ps", name="psq")
                        ps = ps[:, :320]
                        for dk in range(DT):
                            nc.tensor.matmul(
                                ps, lhsT=qkvw[:, dk, ts(do, P)],
                                rhs=hT[:, dk, ts(ncn, 320)],
                                start=(dk == 0), stop=(dk == DT - 1))
                        # split psum head-pair -> per-head base-0 slots
                        t2 = (do - DT) if is_k else do
                        for hh in range(2):
                            slot = (H if is_k else 0) + 2 * t2 + hh
                            dst = qk[:, slot, ts(ncn, 320)]
                            src = ps[hh * 64:hh * 64 + 64, :]
                            scale = 0.125 if is_k else 1.0
                            if qkvb is not None:
                                nc.vector.tensor_scalar(
                                    dst, src,
                                    qkvb[hh * 64:hh * 64 + 64, do:do + 1],
                                    scale, op0=OP.add, op1=OP.mult)
                            elif is_k:
                                nc.vector.tensor_scalar_mul(dst, src, scale)
                            else:
                                nc.vector.tensor_copy(dst, src)

                # v: token-major [tok, h, dh]
                v_sb = vpool.tile([P, NT, H, DH], bf16)
                for t in range(NT):
                    for ncn in range(2):
                        ps = psb.tile([P, 512], f32, tag="ps", name="psv")
                        ps = ps[:, :384]
                        for dk in range(DT):
                            nc.tensor.matmul(
                                ps, lhsT=hT[:, dk, ts(t, P)],
                                rhs=qkvw[:, dk, ds(2 * D + ncn * 384, 384)],
                                start=(dk == 0), stop=(dk == DT - 1))
                        dst = v_sb[:, t, ncn * 6:(ncn + 1) * 6, :]
                        if vb_r is not None:
                            nc.vector.tensor_tensor(
                                dst, ps, vb_r[:, ds(ncn * 384, 384)], OP.add)
                        else:
                            nc.scalar.activation(dst, ps, AF.Copy)

                # ---- attention, one 128-query chunk at a time ----
                o1 = o1pool.tile([P, NT, D], f32)
                for qc in range(NT):
                    E = epool.tile([P, NT, H, P], bf16)   # [k, kt, h, q]
                    for kt in range(NT):
                        for half in range(2):
                            ps_s = pssc.tile([P, 6, P], f32, name="ps_s")
                            for hh in range(6):
                                h = half * 6 + hh
                                nc.tensor.matmul(
                                    ps_s[:, hh, :],
                                    lhsT=qk[:, H + h, ts(kt, P)],
                                    rhs=qk[:, h, ts(qc, P)],
                                    start=True, stop=True)
                            nc.scalar.activation(
                                E[:, kt, half * 6:half * 6 + 6, :],
                                ps_s[:], AF.Exp)
                    S = spool.tile([P, NT, P], f32)
                    nc.vector.tensor_reduce(
                        S, E.rearrange("p kt h q -> p kt q h"),
                        axis=mybir.AxisListType.X, op=OP.add)
                    nc.vector.reciprocal(S, S)
                    nc.vector.tensor_tensor(
                        E, E, S[:, :, None, :].to_broadcast((P, NT, H, P)),
                        OP.mult)

                    av = psav.tile([P, DT, P], f32)
                    for hp in range(DT):
                        for h in (2 * hp, 2 * hp + 1):
                            cb = (h % 2) * 64
                            for kt in range(NT):
                                nc.tensor.matmul(
                                    av[cb:cb + 64, hp, :],
                                    lhsT=v_sb[:, kt, h, :],
                                    rhs=E[:, kt, h, :],
                                    start=(kt == 0), stop=(kt == NT - 1))
                    wa = wapool.tile([P, DT, P], bf16)
                    for hp in range(DT):
                        nc.scalar.activation(wa[:, hp, :], av[:, hp, :],
                                             AF.Copy)

                    rows = P if qc < NT - 1 else LAST_ROWS
                    xs2 = hpool.tile([P, D], f32, tag="xs")
                    if rows < P:
                        nc.vector.memset(xs2, 0.0)
                    nc.sync.dma_start(xs2[:rows, :], x_d[b, ds(qc * P, rows), :])
                    for ncn in range(2):
                        ps = psb.tile([P, 512], f32, tag="ps", name="psp")
                        ps = ps[:, :384]
                        for dk in range(DT):
                            nc.tensor.matmul(
                                ps, lhsT=wa[:, dk, :],
                                rhs=projw[:, dk, ts(ncn, 384)],
                                start=(dk == 0), stop=(dk == DT - 1))
                        dst = o1[:, qc, ts(ncn, 384)]
                        nc.vector.tensor_tensor(
                            dst, xs2[:, ts(ncn, 384)], ps, OP.add)
                        if projb_r is not None:
                            nc.vector.tensor_tensor(
                                dst, dst, projb_r[:, ts(ncn, 384)], OP.add)

                # ---- MLP, one 128-token chunk at a time ----
                h2T = hTpool.tile([P, DT, NP], bf16, tag="hT")
                for t in range(NT):
                    h2_t = hpool.tile([P, D], bf16, tag="h")
                    layer_norm(o1[:, t, :], h2_t, ln2g_r, ln2b_r)
                    for dt in range(DT):
                        nc.sync.dma_start_transpose(
                            h2T[:, dt, ts(t, P)], h2_t[:, ts(dt, P)])

                    gh_t = ghpool.tile([P, HT, P], bf16)
                    for ht in range(HT):
                        ps = psb.tile([P, 512], f32, tag="ps", name="psf")
                        ps = ps[:, :P]
                        for dk in range(DT):
                            nc.tensor.matmul(
                                ps, lhsT=fc1w[:, dk, ts(ht, P)],
                                rhs=h2T[:, dk, ts(t, P)],
                                start=(dk == 0), stop=(dk == DT - 1))
                        if fc1b is not None:
                            nc.scalar.activation(gh_t[:, ht, :], ps, AF.Gelu,
                                                 bias=fc1b[:, ht:ht + 1])
                        else:
                            nc.scalar.activation(gh_t[:, ht, :], ps, AF.Gelu)

                    fo = fopool.tile([P, D], f32)
                    for ncn in range(2):
                        ps = psb.tile([P, 512], f32, tag="ps", name="ps2")
                        ps = ps[:, :384]
                        for kt in range(HT):
                            nc.tensor.matmul(
                                ps, lhsT=gh_t[:, kt, :],
                                rhs=fc2w[:, kt, ts(ncn, 384)],
                                start=(kt == 0), stop=(kt == HT - 1))
                        dst = fo[:, ts(ncn, 384)]
                        nc.vector.tensor_tensor(
                            dst, o1[:, t, ts(ncn, 384)], ps, OP.add)
                        if fc2b_r is not None:
                            nc.vector.tensor_tensor(
                                dst, dst, fc2b_r[:, ts(ncn, 384)], OP.add)
                    rows = P if t < NT - 1 else LAST_ROWS
                    nc.sync.dma_start(out_d[b, ds(t * P, rows), :],
                                      fo[:rows, :])

    nc.compile()
    return nc


def _flags_from(inputs):
    return {
        "qkv_b": bool(np.any(np.asarray(inputs["qkv_b"]) != 0)),
        "fc1_b": bool(np.any(np.asarray(inputs["fc1_b"]) != 0)),
        "proj_b": bool(np.any(np.asarray(inputs["proj_b"]) != 0)),
        "fc2_b": bool(np.any(np.asarray(inputs["fc2_b"]) != 0)),
        "ln1_g": bool(np.any(np.asarray(inputs["ln1_g"]) != 1)),
        "ln1_b": bool(np.any(np.asarray(inputs["ln1_b"]) != 0)),
        "ln2_g": bool(np.any(np.asarray(inputs["ln2_g"]) != 1)),
        "ln2_b": bool(np.any(np.asarray(inputs["ln2_b"]) != 0)),
    }


def build_nc(inputs):
    flags = _flags_from(inputs)
    key = tuple(sorted(flags.items()))
    if key not in _BUILT:
        _BUILT[key] = _build(flags)
    return _BUILT[key]


def make_in_maps(inputs):
    full = {k: np.ascontiguousarray(np.asarray(v, dtype=np.float32))
            for k, v in inputs.items()}
    x = full.pop("x")
    in_maps = []
    for c in range(NCORES):
        m = dict(full)
        m["x"] = np.ascontiguousarray(x[c * BPC:(c + 1) * BPC])
        in_maps.append(m)
    return in_maps


def kernel(**inputs):
    global LAST_EXEC_NS
    from concourse import bass_utils

    nc = build_nc(inputs)
    in_maps = make_in_maps(inputs)
    t0 = time.time()
    r = bass_utils.run_bass_kernel_spmd(nc, in_maps,
                                        core_ids=list(range(NCORES)))
    LAST_EXEC_NS = r.exec_time_ns if r.exec_time_ns else int(
        (time.time() - t0) * 1e9)
    out = np.concatenate([r.results[c]["out"] for c in range(NCORES)], axis=0)
    return out.astype(np.float32)

